# revision 35
# baseline (speedup 1.0000x reference)
"""DirectedDualSAGE (2-layer dual-direction GraphSAGE + MLP head) on 8 trn2
NeuronCores via Bass/Tile.

Sharding: nodes (dsts) block-partitioned 6250/core; each core owns all edges
whose dst lies in its shard, for both edge directions.

v2 design (vs the table-build baseline):
- Gather RAW fp16 source features (128 wide = 256 B rows) instead of fp32
  Wl-transformed ones. mean @ Wl is computed AFTER aggregation (linearity),
  so layer-1 gather tables are pure host-prepared ExternalInputs (no device
  build at all) and layer 2 needs a single fp16 AllGather of x2 (one
  collective total, 4x less traffic than two fp32 ones).
- All dma_gathers use prepare_only=True + trigger_dma so the gpsimd engine
  only generates descriptors (~0.34 ns/row) and never blocks on DMA drain;
  consumers wait on the DMA-completion semaphores via Tile's DMASW ticks.
- Segment-mean accumulation: dsts sorted by descending degree per
  (direction, src-half); round r occupies slot prefix [0, n_r). Round 0 is
  gathered directly into the accumulator (copy-init for free); later rounds
  go through staging tiles + one contiguous vector add per fragment, fp16.
- Un-permutation to natural dst order via an SBUF-source transposing
  dma_gather straight out of the accumulator (no DRAM roundtrip), which
  also delivers the mean FEATURE-major - exactly the moving operand the
  h-pass matmuls want. recip (1/deg) is folded per half before unpermuting.
- Dense passes all fp16 inputs into fp32 PSUM: h = relu(Wr.T@xT + Wl.T@meanT
  + b) is 4 matmuls per 512-col segment, no PE transposes, no PSUM copies.

kernel(**inputs) takes full unsharded inputs, returns the full [N] output.
"""
import numpy as np

import concourse.bacc as bacc
import concourse.tile as tile
import concourse.mybir as mybir
from concourse import bass_utils

F32 = mybir.dt.float32
F16 = mybir.dt.float16
I16 = mybir.dt.int16

N = 50000
NC = 8
NLOC = N // NC            # 6250
NLOCP = 6272              # 49*128
NCH = NLOCP // 128        # 49 chunks
HALF = 25000              # src half split (int16 idx range)
ZHEAD = 128               # zero rows at table head
HROWS = ZHEAD + HALF + 176     # 25304 rows per half table
TROWS = ZHEAD + N + 176        # 50304 rows for the AllGather'ed x2 table
BZERO = ZHEAD + HALF      # 25128: B-half zero idx (tail / y2 row 50128)
SMAX = 2560               # max rows per dma_gather call
NQ = 4                    # SWDGE queues

_CACHE = {}
DEBUG = False
PREP = False  # use prepare_only + trigger_dma for gathers
G0_SPLIT = True      # split round-0 gather across queues
UNPERM_SPLIT = False  # split unperm gathers across queues


# ----------------------------------------------------------------- host prep

def _round_up(v, m):
    return (v + m - 1) // m * m


def _per_core_half(src, dst, half_mask):
    out = []
    for c in range(NC):
        m = (dst // NLOC == c) & half_mask
        s = src[m]
        dloc = (dst[m] - c * NLOC).astype(np.int64)
        deg = np.bincount(dloc, minlength=NLOCP).astype(np.int64)
        perm = np.argsort(-deg, kind="stable").astype(np.int64)
        pos = np.empty(NLOCP, dtype=np.int64)
        pos[perm] = np.arange(NLOCP)
        order = np.argsort(dloc, kind="stable")
        sd = dloc[order]
        ss = s[order]
        if len(sd):
            starts = np.r_[0, 1 + np.flatnonzero(np.diff(sd))]
            group_id = np.zeros(len(sd), dtype=np.int64)
            group_id[starts[1:]] = 1
            group_id = np.cumsum(group_id)
            rank = np.arange(len(sd)) - starts[group_id]
        else:
            rank = sd
        slot = pos[sd]
        maxdeg = int(deg.max()) if len(sd) else 0
        rounds = []
        for r in range(maxdeg):
            mr = rank == r
            rounds.append((int(np.count_nonzero(mr)), slot[mr], ss[mr]))
        out.append(dict(deg=deg, pos=pos, rounds=rounds))
    return out


def _preprocess(edge_index_in, edge_index_out):
    plan = {"dirs": {}}
    for dname, ei in (("in", edge_index_in), ("out", edge_index_out)):
        src = ei[0].astype(np.int64)
        dst = ei[1].astype(np.int64)
        dinfo = {"halves": {}, "recip": []}
        for c in range(NC):
            m = dst // NLOC == c
            dloc = dst[m] - c * NLOC
            cnt = np.bincount(dloc, minlength=NLOCP).astype(np.float32)
            dinfo["recip"].append((1.0 / np.maximum(cnt, 1.0)).astype(np.float32))
        for hname, is_a in (("A", True), ("B", False)):
            half_mask = (src < HALF) if is_a else (src >= HALF)
            cores = _per_core_half(src, dst, half_mask)
            nrounds = max(len(ci["rounds"]) for ci in cores)
            NR = []
            for r in range(nrounds):
                mx = max((ci["rounds"][r][0] if r < len(ci["rounds"]) else 0)
                         for ci in cores)
                NR.append(_round_up(max(mx, 1), 128))
            NR[0] = NLOCP  # full first round: gathered directly into the acc
            zi = 0 if is_a else BZERO
            streams = []
            for ci in cores:
                parts = []
                for r in range(nrounds):
                    vec = np.full(NR[r], zi, dtype=np.int64)
                    if r < len(ci["rounds"]):
                        _, slots, ss = ci["rounds"][r]
                        vec[slots] = (ss + ZHEAD) if is_a else (ss - HALF + ZHEAD)
                    parts.append(vec)
                stream = np.concatenate(parts)
                assert stream.max(initial=0) < 32768
                streams.append(stream.astype(np.int16))
            # group 0 = round 0 exactly (direct gather into the acc);
            # then uniform SMAX-row cuts over rounds >= 1 (fragments add
            # into acc slot range [s0, s1))
            L = int(sum(NR))
            groups = [(0, [(0, 0, NLOCP, 0)])]
            r, r_off = 1, 0
            off = NLOCP
            while off < L:
                rows = min(SMAX, L - off)
                frags = []
                done = 0
                while done < rows:
                    take = min(NR[r] - r_off, rows - done)
                    frags.append((done, r_off, take, r))
                    done += take
                    r_off += take
                    if r_off == NR[r]:
                        r += 1
                        r_off = 0
                groups.append((off, frags))
                off += rows
            dinfo["halves"][hname] = dict(
                NR=NR, L=int(sum(NR)), streams=streams, groups=groups,
                unperm=[ci["pos"].astype(np.int16) for ci in cores],
                perm=[np.argsort(ci["pos"]).astype(np.int64) for ci in cores],
                is_a=is_a,
            )
        plan["dirs"][dname] = dinfo
    return plan


def _wrap_idx(idx):
    L = idx.shape[0]
    assert L % 16 == 0
    w = idx.reshape(L // 16, 16).T.astype(np.int16)
    return np.ascontiguousarray(np.tile(w, (8, 1)))


# ------------------------------------------------------------- device program

def _build_program(plan):
    nc = bacc.Bacc("TRN2", target_bir_lowering=False, debug=False,
                   num_devices=NC, num_swdge_queues=NQ)
    dims = ("in", "out")
    inp = {}

    def dram_in(name, shape, dt=F32):
        inp[name] = nc.dram_tensor(name, list(shape), dt, kind="ExternalInput")
        return inp[name]

    xt16 = {h: dram_in(f"xt16_{h}", [HROWS, 128], F16) for h in ("A", "B")}
    dram_in("xt_loc16", [128, NLOCP], F16)
    dram_in("ident16", [128, 128], F16)
    for li in (1, 2):
        dram_in(f"wr_pair{li}", [128, 128], F16)
        dram_in(f"bias_pk{li}", [128, 1])
        dram_in(f"wcx{li}", [128, 128], F16)
        dram_in(f"wch{li}", [128, 128], F16)
        dram_in(f"cb{li}", [128, 1])
    dram_in("wl_pair2", [128, 128], F16)
    dram_in("fw16", [128, 1], F16)
    dram_in("fb", [1, 1])
    for d in dims:
        for h in ("A", "B"):
            L = plan["dirs"][d]["halves"][h]["L"]
            dram_in(f"stream_{d}_{h}", [128, L // 16], I16)
            dram_in(f"unperm_{d}_{h}", [128, NLOCP // 16], I16)
            dram_in(f"recip_{d}_{h}", [128, NCH], F16)

    y2t = nc.dram_tensor("y2t", [TROWS, 128], F16, kind="Internal",
                         addr_space="Shared")
    y2sl = nc.dram_tensor("y2sl", [NLOCP, 128], F16, kind="Internal")
    accd = {d: nc.dram_tensor(f"accd_{d}", [2, NLOCP, 128], F16,
                              kind="Internal") for d in dims}
    out_t = nc.dram_tensor("out", [1, NLOC], F32, kind="ExternalOutput")

    _qctr = [0]

    def next_queue():
        q = _qctr[0] % NQ
        _qctr[0] += 1
        return q

    RELU = mybir.ActivationFunctionType.Relu
    COPY = mybir.ActivationFunctionType.Copy

    with tile.TileContext(nc) as tc:
        qsems = [nc.alloc_semaphore(f"swdge_dma_q{q}") for q in range(NQ)]
        with tc.tile_pool(name="const", bufs=1) as cpool, \
             tc.tile_pool(name="idxp", bufs=1) as idxp, \
             tc.tile_pool(name="idxg", bufs=8) as idxgp, \
             tc.tile_pool(name="feat", bufs=2) as featp, \
             tc.tile_pool(name="xres", bufs=1) as xresp, \
             tc.tile_pool(name="accp", bufs=3) as accp, \
             tc.tile_pool(name="natp", bufs=5) as natp, \
             tc.tile_pool(name="stg", bufs=10) as stgp, \
             tc.tile_pool(name="ystg", bufs=2) as ystgp, \
             tc.tile_pool(name="ps", bufs=3, space="PSUM") as psp, \
             tc.tile_pool(name="psf", bufs=1, space="PSUM") as psfp, \
             tc.tile_pool(name="psy", bufs=2, space="PSUM") as psyp:

            def load_const(name, shape, dt=F32):
                t = cpool.tile(list(shape), dt, tag=name, name=f"c_{name}")
                nc.sync.dma_start(t[:], inp[name][tuple(slice(None) for _ in shape)])
                return t

            ident_t = load_const("ident16", [128, 128], F16)
            W = {}
            for li in (1, 2):
                for nm, shp, dt in (("wr_pair", [128, 128], F16),
                                    ("bias_pk", [128, 1], F32),
                                    ("wcx", [128, 128], F16),
                                    ("wch", [128, 128], F16),
                                    ("cb", [128, 1], F32)):
                    W[f"{nm}{li}"] = load_const(f"{nm}{li}", shp, dt)
            W["wl_pair2"] = load_const("wl_pair2", [128, 128], F16)
            fw_t = load_const("fw16", [128, 1], F16)
            fb_t = load_const("fb", [1, 1])
            recip_t = {}
            unperm_t = {}
            for d in dims:
                for h in ("A", "B"):
                    recip_t[d, h] = load_const(f"recip_{d}_{h}", [128, NCH], F16)
                    ut = idxp.tile([128, NLOCP // 16], I16, tag=f"up_{d}_{h}",
                                   name=f"up_{d}_{h}")
                    nc.sync.dma_start(ut[:], inp[f"unperm_{d}_{h}"][:, :])
                    unperm_t[d, h] = ut

            zero_t = cpool.tile([128, 128], F16, tag="zero")
            nc.vector.memset(zero_t[:], 0.0)

            def zero_rows(start):
                nc.sync.dma_start(
                    y2t[start:start + 128, :]
                    .rearrange("(k p) c -> p k c", p=128),
                    zero_t[:].rearrange("p (k c) -> p k c", k=1))

            zero_rows(0)
            zero_rows(ZHEAD + N)
            zero_rows(TROWS - 128)

            # resident local features (fp16, feature-major)
            xt_t = xresp.tile([128, NLOCP], F16, tag="xt", name="xt_loc")
            nc.sync.dma_start(xt_t[:], inp["xt_loc16"][:, :])

            # ---------------- gather + accumulate machinery
            def prep_gather(out_ap, tab_ap, gidx_ap, rows):
                q = next_queue()
                if PREP:
                    nc.gpsimd.dma_gather(
                        out_ap, tab_ap, gidx_ap,
                        num_idxs=rows, num_idxs_reg=rows,
                        elem_size=128, elem_step=128, single_packet=False,
                        prepare_only=True, sem=qsems[q], queue_num=q)
                    nc.gpsimd.trigger_dma(count=None, queue_num=q)
                else:
                    nc.gpsimd.dma_gather(
                        out_ap, tab_ap, gidx_ap,
                        num_idxs=rows, num_idxs_reg=rows,
                        elem_size=128, elem_step=128, single_packet=False,
                        queue_num=q)

            # 4-way chunk split: a solo gather runs at single-queue speed
            # (~107 rows/us); splitting across the 4 queues restores ~430.
            QSPLIT = [(0, 13), (13, 12), (25, 12), (37, 12)]

            def gather_half(li, d, h, tab_ap, deferred=None):
                """Gather+accumulate one (dir, half); returns acc tile.
                `deferred` thunks (previous stream's unperm gathers) are
                emitted after this stream's second group so they don't
                stall the engine at the stream boundary."""
                hinfo = plan["dirs"][d]["halves"][h]
                acc = accp.tile([128, NCH, 128], F16, tag="acc",
                                name=f"acc{li}_{d}_{h}")
                ngroups = len(hinfo["groups"])
                for gi, (goff, frags) in enumerate(hinfo["groups"]):
                    rows = sum(f[2] for f in frags)
                    gidx = idxgp.tile([128, NLOCP // 16], I16, tag="gidx")
                    nc.scalar.dma_start(
                        gidx[:, 0:rows // 16],
                        inp[f"stream_{d}_{h}"][:, goff // 16:(goff + rows) // 16])
                    if goff == 0:
                        splits = QSPLIT if G0_SPLIT else [(0, NCH)]
                        for c0, cn in splits:
                            prep_gather(
                                acc[:, c0:c0 + cn, :], tab_ap,
                                gidx[:, c0 * 8:(c0 + cn) * 8], cn * 128)
                        continue
                    stg = stgp.tile([128, SMAX // 128, 128], F16, tag="stg")
                    if gi == ngroups - 1 and rows >= 512:
                        # split the tail group across all 4 queues so the
                        # stream's drain (on the unperm critical path) is
                        # ~4x shorter
                        nr4 = rows // 4 // 128 * 128
                        cuts = [nr4, nr4, nr4, rows - 3 * nr4]
                        o = 0
                        for cr in cuts:
                            if cr:
                                prep_gather(
                                    stg[:, o // 128:(o + cr) // 128, :], tab_ap,
                                    gidx[:, o // 16:(o + cr) // 16], cr)
                            o += cr
                    else:
                        prep_gather(stg[:, 0:rows // 128, :], tab_ap,
                                    gidx[:, 0:rows // 16], rows)
                    for stg_off, slot_off, nrows, r in frags:
                        cr = nrows // 128
                        c0 = slot_off // 128
                        s_ap = stg[:, stg_off // 128:stg_off // 128 + cr, :]
                        a_ap = acc[:, c0:c0 + cr, :]
                        nc.vector.tensor_add(a_ap, a_ap, s_ap)
                    if gi == 3 and deferred:
                        while deferred:
                            deferred.pop(0)()
                # fold recip (node-major, per-half: recip is linear)
                rb = recip_t[d, h][:].unsqueeze(2).broadcast_to((128, NCH, 128))
                nc.vector.tensor_mul(acc[:], acc[:], rb)
                return acc

            hidx = {"A": 0, "B": 1}

            def unperm_half(d, h, out_tile):
                """4-way-split non-transpose gather accd -> natural node
                order, node-major [128, NCH, 128]."""
                for c0, cn in QSPLIT:
                    prep_gather(
                        out_tile[:, c0:c0 + cn, :], accd[d][hidx[h], :, :],
                        unperm_t[d, h][:, c0 * 8:(c0 + cn) * 8], cn * 128)

            def agg_dir(li, d, tab_fn, deferred, after):
                """Full aggregation for one direction -> mean_nat
                [128, NCH, 128] fp16, node-major, natural order (columns
                0:64 = transformed in-mean, 64:128 = out-mean; only the
                d-half is meaningful). Appends its own unperm+merge work
                to `after` as deferred thunks."""
                mn = natp.tile([128, NCH, 128], F16, tag="nat",
                               name=f"mn{li}_{d}")
                ms = natp.tile([128, NCH, 128], F16, tag="nat")
                acc_a = gather_half(li, d, "A", tab_fn(d, "A"), deferred)
                nc.sync.dma_start(
                    accd[d][0, :, :].rearrange("(c p) f -> p c f", p=128),
                    acc_a[:])
                acc_b = gather_half(li, d, "B", tab_fn(d, "B"))
                nc.sync.dma_start(
                    accd[d][1, :, :].rearrange("(c p) f -> p c f", p=128),
                    acc_b[:])

                def fin():
                    unperm_half(d, "A", mn)
                    unperm_half(d, "B", ms)
                    nc.vector.tensor_add(mn[:], mn[:], ms[:])
                after.append(fin)
                return mn

            def seg_widths():
                segs = []
                off = 0
                while off < NLOCP:
                    w = min(512, NLOCP - off)
                    segs.append((off, w))
                    off += w
                return segs

            def h_pass_half(li, side, feat_t, mn, h_t):
                """One direction's h half: relu(Wr.T @ xT + mean.T + b).
                The mean transpose rides the PE into the same PSUM via
                identity matmuls. Emitted per direction as soon as that
                direction's mean is merged, overlapping the other
                direction's gathers."""
                lo, hi = (0, 64) if side == "in" else (64, 128)
                tp = None if side == "in" else (0, 64)
                for off, w in seg_widths():
                    ps = psp.tile([128, 512], F32, tag="ps")
                    nc.tensor.matmul(ps[lo:hi, 0:w],
                                     W[f"wr_pair{li}"][:, lo:hi],
                                     feat_t[:, off:off + w],
                                     start=True, stop=False, tile_position=tp)
                    nch = w // 128
                    for k in range(nch):
                        c = (off + 128 * k) // 128
                        nc.tensor.matmul(ps[lo:hi, 128 * k:128 * (k + 1)],
                                         mn[:, c, lo:hi], ident_t[:],
                                         start=False, stop=k == nch - 1,
                                         tile_position=tp)
                    nc.scalar.activation(h_t[lo:hi, off:off + w],
                                         ps[lo:hi, 0:w], RELU,
                                         bias=W[f"bias_pk{li}"][lo:hi, 0:1])

            def hcomb_pass(li, feat_t, mn_out, h_t, out_cb):
                """Segment-interleaved: out-half h (needs the last-arriving
                out mean) immediately followed by comb for that segment, so
                the last comb lands ~one segment after the last h."""
                for off, w in seg_widths():
                    ps = psp.tile([128, 512], F32, tag="ps")
                    nc.tensor.matmul(ps[64:128, 0:w],
                                     W[f"wr_pair{li}"][:, 64:128],
                                     feat_t[:, off:off + w],
                                     start=True, stop=False,
                                     tile_position=(0, 64))
                    nch = w // 128
                    for k in range(nch):
                        c = (off + 128 * k) // 128
                        nc.tensor.matmul(ps[64:128, 128 * k:128 * (k + 1)],
                                         mn_out[:, c, 64:128], ident_t[:],
                                         start=False, stop=k == nch - 1,
                                         tile_position=(0, 64))
                    nc.scalar.activation(h_t[64:128, off:off + w],
                                         ps[64:128, 0:w], RELU,
                                         bias=W[f"bias_pk{li}"][64:128, 0:1])
                    ps2 = psp.tile([128, 512], F32, tag="ps")
                    nc.tensor.matmul(ps2[:, 0:w], W[f"wcx{li}"][:],
                                     feat_t[:, off:off + w],
                                     start=True, stop=False)
                    nc.tensor.matmul(ps2[:, 0:w], W[f"wch{li}"][:],
                                     h_t[:, off:off + w], start=False, stop=True)
                    out_cb(off, w, ps2)

            # ---------------- layer 1 (tables are host-prepared inputs:
            # row i = [x[i] @ Wl1_in | x[i] @ Wl1_out] fp16)
            def l1_tab(d, h):
                return xt16[h][0:HROWS, :]

            af_in, af_out = [], []
            mn1_in = agg_dir(1, "in", l1_tab, None, af_in)
            mn1_out = agg_dir(1, "out", l1_tab, af_in, af_out)
            h1_t = featp.tile([128, NLOCP], F16, tag="bigfeat")
            h_pass_half(1, "in", xt_t, mn1_in, h1_t)
            while af_out:
                af_out.pop(0)()
            x2_t = featp.tile([128, NLOCP], F16, tag="bigfeat")

            # comb1 -> x2, with the y2 table rows (node-major transformed
            # pairs) built per segment; one AllGather right after the
            # last segment. (Chunked collectives are impossible: the
            # rank-concat output AP for a row range is non-contiguous.)
            def fire_cc(upto):
                if upto >= NLOC:
                    nc.gpsimd.collective_compute(
                        "AllGather", mybir.AluOpType.bypass,
                        replica_groups=[list(range(NC))],
                        ins=[y2sl[0:NLOC, :]],
                        outs=[y2t[ZHEAD:ZHEAD + N, :]],
                    )

            def l1_out(off, w, ps):
                nc.scalar.activation(x2_t[:, off:off + w], ps[:, 0:w], RELU,
                                     bias=W["cb1"][:])
                ncc = w // 128
                ps2 = psyp.tile([128, 512], F32, tag="psy")
                for k in range(ncc):
                    nc.tensor.matmul(ps2[:, 128 * k:128 * (k + 1)],
                                     x2_t[:, off + 128 * k:off + 128 * (k + 1)],
                                     W["wl_pair2"][:], start=True, stop=True)
                ys = ystgp.tile([128, 512], F16, tag="ys")
                nc.scalar.activation(ys[:, 0:w], ps2[:, 0:w], COPY)
                nc.sync.dma_start(
                    y2sl[off:off + w, :].rearrange("(k p) c -> p k c", p=128),
                    ys[:, 0:w].rearrange("p (k c) -> p k c", k=ncc))
                fire_cc(min(off + w, NLOC))
            hcomb_pass(1, xt_t, mn1_out, h1_t, l1_out)

            # ---------------- layer 2
            def l2_tab(d, h):
                if h == "A":
                    return y2t[0:HROWS, :]
                return y2t[HALF:TROWS, :]

            af2_in, af2_out = [], []
            mn2_in = agg_dir(2, "in", l2_tab, None, af2_in)
            mn2_out = agg_dir(2, "out", l2_tab, af2_in, af2_out)
            h2_t = featp.tile([128, NLOCP], F16, tag="bigfeat")
            h_pass_half(2, "in", x2_t, mn2_in, h2_t)
            while af2_out:
                af2_out.pop(0)()

            def l2_out(off, w, ps):
                x3 = ystgp.tile([128, 512], F16, tag="x3")
                nc.scalar.activation(x3[:, 0:w], ps[:, 0:w], RELU,
                                     bias=W["cb2"][:])
                psf = psfp.tile([1, 512], F32, tag="psf")
                nc.tensor.matmul(psf[0:1, 0:w], fw_t[:], x3[:, 0:w],
                                 start=True, stop=True)
                osb = ystgp.tile([1, 512], F32, tag="osb")
                nc.vector.tensor_scalar_add(osb[0:1, 0:w],
                                            psf[0:1, 0:w], fb_t[0:1, 0:1])
                wv = min(w, NLOC - off)
                if wv > 0:
                    nc.sync.dma_start(out_t[0:1, off:off + wv], osb[0:1, 0:wv])
            hcomb_pass(2, x2_t, mn2_out, h2_t, l2_out)

    nc.compile()
    return nc


# ------------------------------------------------------------------ interface

def _make_in_maps(plan, inputs):
    x = np.asarray(inputs["x"], dtype=np.float32)
    x16 = x.astype(np.float16)
    # layer-1 gather tables: row i = [x[i] @ Wl1_in | x[i] @ Wl1_out] fp16
    wl1 = np.concatenate([np.asarray(inputs["in_Wl0"], np.float32),
                          np.asarray(inputs["out_Wl0"], np.float32)], axis=1)
    y1 = (x @ wl1).astype(np.float16)
    xt16_A = np.zeros((HROWS, 128), dtype=np.float16)
    xt16_A[ZHEAD:ZHEAD + HALF] = y1[0:HALF]
    xt16_B = np.zeros((HROWS, 128), dtype=np.float16)
    xt16_B[ZHEAD:ZHEAD + HALF] = y1[HALF:N]
    ident16 = np.eye(128, dtype=np.float16)

    def f16(a):
        return np.ascontiguousarray(np.asarray(a, np.float32).astype(np.float16))

    def cat16(a, b):
        return np.ascontiguousarray(
            np.concatenate([np.asarray(a, np.float32), np.asarray(b, np.float32)],
                           axis=1).astype(np.float16))

    wkeys = [("in_Wl0", "out_Wl0", "in_Wr0", "out_Wr0", "in_bl0", "out_bl0",
              "comb_W0", "comb_b0"),
             ("in_Wl1", "out_Wl1", "in_Wr1", "out_Wr1", "in_bl1", "out_bl1",
              "comb_W1", "comb_b1")]
    common = {
        "xt16_A": xt16_A,
        "xt16_B": xt16_B,
        "ident16": ident16,
        "wl_pair2": cat16(inputs["in_Wl1"], inputs["out_Wl1"]),
        "fw16": f16(np.asarray(inputs["final_W"], np.float32).reshape(128, 1)),
        "fb": np.asarray(inputs["final_b"], np.float32).reshape(1, 1).copy(),
    }
    for li, (wl_i, wl_o, wr_i, wr_o, bl_i, bl_o, cw, cb) in enumerate(wkeys, 1):
        common[f"wr_pair{li}"] = cat16(inputs[wr_i], inputs[wr_o])
        common[f"bias_pk{li}"] = np.concatenate(
            [np.asarray(inputs[bl_i], np.float32),
             np.asarray(inputs[bl_o], np.float32)])[:, None].copy()
        cwf = np.asarray(inputs[cw], np.float32)
        common[f"wcx{li}"] = np.ascontiguousarray(cwf[0:128].astype(np.float16))
        common[f"wch{li}"] = np.ascontiguousarray(cwf[128:256].astype(np.float16))
        common[f"cb{li}"] = np.asarray(inputs[cb], np.float32)[:, None].copy()

    in_maps = []
    for c in range(NC):
        m = dict(common)
        xl = np.zeros((128, NLOCP), dtype=np.float16)
        xl[:, :NLOC] = x16.T[:, c * NLOC:(c + 1) * NLOC]
        m["xt_loc16"] = xl
        for d in ("in", "out"):
            dinfo = plan["dirs"][d]
            r = dinfo["recip"][c]  # [NLOCP] natural order
            for h in ("A", "B"):
                hinfo = dinfo["halves"][h]
                m[f"stream_{d}_{h}"] = _wrap_idx(hinfo["streams"][c])
                m[f"unperm_{d}_{h}"] = _wrap_idx(hinfo["unperm"][c])
                # recip in slot-major order for this half's permutation
                rp = r[hinfo["perm"][c]].astype(np.float16)  # slot s -> node
                m[f"recip_{d}_{h}"] = np.ascontiguousarray(
                    rp.reshape(NCH, 128).T)
        in_maps.append(m)
    return in_maps


def kernel(**inputs):
    plan = _preprocess(np.asarray(inputs["edge_index_in"]),
                       np.asarray(inputs["edge_index_out"]))
    key = tuple(
        (d, h, tuple(plan["dirs"][d]["halves"][h]["NR"]))
        for d in ("in", "out") for h in ("A", "B"))
    if key not in _CACHE:
        _CACHE[key] = _build_program(plan)
    nc = _CACHE[key]
    in_maps = _make_in_maps(plan, inputs)
    res = bass_utils.run_bass_kernel_spmd(nc, in_maps, core_ids=list(range(NC)))
    out = np.concatenate([r["out"][0] for r in res.results])
    return out.astype(np.float32)
